# revision 32
# baseline (speedup 1.0000x reference)
"""Binarized 2-layer conv net (BinaryConv2d -> BinaryTanh -> BinaryConv2d -> Scale)
for Trainium2, data-parallel over the batch dim across 8 NeuronCores.

Math (matching the reference):
    h   = conv2d(x, sign(w1), pad=1) + sign(b1)
    h   = sign(h)                       # sign(clip(h,-1,1)) == sign(h)
    out = (conv2d(h, sign(w2), pad=1) + sign(b2)) * scale

Per-core design (8 images):
  * x is hi/lo fp16 split (hi+lo ~ 24 mantissa bits, effectively fp32-exact)
    folded into the matmul K dim: partitions g*64 + {0:32 hi, 32:64 lo} for
    row-group g; weights duplicated across hi/lo halves. One copy of x in
    SBUF per image ([128, 34, 66] fp16, 2 row-groups of 32 output rows each
    with halo rows), 9 conv taps read via shifted access patterns.
  * conv1: K=64, M=64, 9 tap-matmuls per 512-px block, 4 blocks concurrent
    on the four 64x64 PE tiles. Evacuation is VectorE (psum + b1) >= 0 into
    the fp8 h staging buffer as h01 in {0,1} (0.5 at padding borders), so
    ScalarE stays off the critical path; conv2 compensates exactly with
    out = 2*psum - sum(w2 signs) + sign(b2).
  * h staging: [128 = 2 row-groups x 64 ch, 35 rows x 80 cols] fp8e4 with
    0.5 borders + halo rows; 80B row pitch makes dy-shifted tap pairs
    16B-aligned for DoubleRow.
  * conv2: fp8e4 DoubleRow - per dx, taps (dy0,dy1) form one double-row
    matmul and (dy2, zero-weights) a second: 6 half-rate matmuls per block
    instead of 9 full-rate. {0,0.5,1} are exact in fp8e4, products
    accumulate in fp32 psum, so conv2 is exact. DoubleRow outputs must land
    at psum partition base 0 (col tiling is ISA-incompatible with DR), so
    each block gets its own [64, 512] psum tile; concurrency comes from the
    two K row-groups, with same-weight matmuls adjacent for LDW elision.
    Per round, the four evacuations (ScalarE, scale=2, bias) fill one
    [64, 2048] tile written by a single 4D-AP output DMA.
  * Software pipeline: iteration i emits conv1(i) then conv2(i-1), with the
    x DMA for image i+1 prefetched at the top - conv2(i-1)'s dependencies
    (conv1(i-1) evacuations + halo DMAs) complete while the PE runs
    conv1(i), so the PE never stalls at the conv1->conv2 boundary.
  * Output: psum + sign(b2) are exact small integers -> written as fp16
    (exact to 2048), DMA'd at half the bytes; host multiplies by scale in
    fp32 (bit-identical to doing it on-device).
"""

import numpy as np
import ml_dtypes

import concourse.mybir as mybir
import concourse.tile as tile
from concourse import bacc
from concourse.ap import AP
from concourse.bass_utils import run_bass_kernel_spmd

F32 = mybir.dt.float32
F16 = mybir.dt.float16
FP8 = mybir.dt.float8e4
ACT_SIGN = mybir.ActivationFunctionType.Sign
ACT_IDENT = mybir.ActivationFunctionType.Identity
ALU_MULT = mybir.AluOpType.mult
ALU_ADD = mybir.AluOpType.add
ALU_GE = mybir.AluOpType.is_ge
DR = mybir.MatmulPerfMode.DoubleRow

N_CORES = 8
IMGS_PER_CORE = 8
CIN, COUT = 32, 64
H = W = 64
XROWS, XCOLS = 34, 66          # per-group x slab: 32 out rows + 2 halo
XFREE = XROWS * XCOLS          # 2244
HROWS, HP = 35, 80             # h slab: 34 rows + 1 zero overrun, 80B pitch
HFREE = HROWS * HP             # 2800


def build_nc(reps: int = 1, nbuf: int = 2, dr: bool = True) -> bacc.Bacc:
    nc = bacc.Bacc("TRN2", target_bir_lowering=False)

    xg_t = nc.dram_tensor("xg", [IMGS_PER_CORE, 128, XFREE], F16, kind="ExternalInput")
    w1s_t = nc.dram_tensor("w1s", [128, 9 * 64], F16, kind="ExternalInput")
    w2s_t = nc.dram_tensor("w2s", [128, 768], FP8, kind="ExternalInput")
    b1s_t = nc.dram_tensor("b1s", [128, 1], F32, kind="ExternalInput")
    b2s_t = nc.dram_tensor("b2s", [128, 1], F32, kind="ExternalInput")
    out_t = nc.dram_tensor("out", [IMGS_PER_CORE, COUT, H * W], F16, kind="ExternalOutput")

    with tile.TileContext(nc) as tc:
        w1 = nc.alloc_sbuf_tensor("w1sb", [128, 9 * 64], F16).ap()
        w2 = nc.alloc_sbuf_tensor("w2sb", [128, 768], FP8).ap()
        b1 = nc.alloc_sbuf_tensor("b1sb", [128, 1], F32).ap()
        b2 = nc.alloc_sbuf_tensor("b2sb", [128, 1], F32).ap()
        xs = [nc.alloc_sbuf_tensor(f"xs_{b}", [128, XFREE], F16).ap()
              for b in range(nbuf)]
        hs = [nc.alloc_sbuf_tensor(f"hs_{b}", [128, HFREE], FP8).ap()
              for b in range(nbuf)]

        # x loads split at row 18 (round 0 of conv1 reads rows 0..17, round 1
        # reads 16..33) so the first matmuls only wait for half the 574KB.
        XSPLIT = 18 * XCOLS

        def load_x(iv):
            # First half rides the Activation HWDGE ring (ScalarE-issued),
            # second half the SP ring - the two complete in parallel, so
            # round 0's matmuls (gated on the first half) start sooner.
            buf, img = iv % nbuf, iv % IMGS_PER_CORE
            nc.scalar.dma_start(out=xs[buf][:, :XSPLIT],
                                in_=xg_t.ap()[img][:, :XSPLIT])
            nc.sync.dma_start(out=xs[buf][:, XSPLIT:],
                              in_=xg_t.ap()[img][:, XSPLIT:])

        # Weights first (land fast, gate the first Ldweights), then the
        # first x image; x rides the parallel Activation ring. Tap-0 of w1
        # ships separately (16KB) so the very first Ldweights doesn't wait
        # for the full 147KB w1 transfer.
        nc.sync.dma_start(out=w1[:, 0:64], in_=w1s_t.ap()[:, 0:64])
        nc.sync.dma_start(out=w1[:, 64:], in_=w1s_t.ap()[:, 64:])
        nc.sync.dma_start(out=w2, in_=w2s_t.ap())
        nc.sync.dma_start(out=b1, in_=b1s_t.ap())
        nc.sync.dma_start(out=b2, in_=b2s_t.ap())
        load_x(0)
        # h is stored as h01 = (conv1+b1 >= 0) in {0,1}; padding borders hold
        # 0.5 so that h = 2*h01 - 1 is 0 there, and conv2 compensates with
        # out = 2*psum - sum(w2) + sign(b2). This lets conv1 evacuate via a
        # single DVE add/is_ge instead of ScalarE Sign, taking the Activation
        # engine off the real-HW critical path. Border-only memsets (rows
        # 0/33/34, cols 0/65): the interior is overwritten every image and
        # the 66:80 column pad is never read.
        for b in range(nbuf):
            hvb = hs[b].rearrange("p (r c) -> p r c", c=HP)
            nc.vector.memset(hvb[:, 0:1, :], 0.5)
            nc.vector.memset(hvb[:, 33:35, :], 0.5)
            nc.vector.memset(hvb[:, :, 0:1], 0.5)
            nc.vector.memset(hvb[:, :, 65:66], 0.5)

        w2v = w2.rearrange("p (dx pair plane m) -> p dx pair plane m",
                           dx=3, pair=2, plane=2)

        with tc.tile_pool(name="ps1", bufs=2, space="PSUM") as pool1, \
             tc.tile_pool(name="ps2", bufs=2, space="PSUM") as pool2, \
             tc.tile_pool(name="ob", bufs=4) as ob_pool:

            def emit_conv1(iv):
                buf = iv % nbuf
                xv = xs[buf].rearrange("p (r c) -> p r c", c=XCOLS)
                hv = hs[buf].rearrange("p (r c) -> p r c", c=HP)
                tb = ob_pool.tile([128, 1024], FP8, tag="tb", name="tb")
                for r in range(2):
                    bA, bB = 2 * r, 2 * r + 1   # block = 8 output rows
                    pc = pool1.tile([128, 512], F32, tag="c1a", name="pc")
                    pd = pool1.tile([128, 512], F32, tag="c1b", name="pd")
                    for tap in range(9):
                        dy, dx = divmod(tap, 3)
                        st, sp = tap == 0, tap == 8
                        lw0 = w1[0:64, tap * 64:(tap + 1) * 64]
                        lw1 = w1[64:128, tap * 64:(tap + 1) * 64]
                        rA0 = xv[0:64, 8 * bA + dy: 8 * bA + dy + 8, dx: dx + 64]
                        rA1 = xv[64:128, 8 * bA + dy: 8 * bA + dy + 8, dx: dx + 64]
                        rB0 = xv[0:64, 8 * bB + dy: 8 * bB + dy + 8, dx: dx + 64]
                        rB1 = xv[64:128, 8 * bB + dy: 8 * bB + dy + 8, dx: dx + 64]
                        nc.tensor.matmul(pc[0:64, :], lw0, rA0, start=st, stop=sp,
                                         tile_position=(0, 0), skip_group_check=True)
                        nc.tensor.matmul(pc[64:128, :], lw1, rA1, start=st, stop=sp,
                                         tile_position=(64, 64), skip_group_check=True)
                        nc.tensor.matmul(pd[64:128, :], lw0, rB0, start=st, stop=sp,
                                         tile_position=(0, 64), skip_group_check=True)
                        nc.tensor.matmul(pd[0:64, :], lw1, rB1, start=st, stop=sp,
                                         tile_position=(64, 0), skip_group_check=True)
                    # h[g, out rows of block, 1:65] = sign(psum + b1).
                    # pc halves are group-aligned -> one direct ACT. pd is
                    # crosswise (tiles (0,64)/(64,0) force the psum column
                    # half) -> ACT into a temp, then two partition-crossing
                    # SBUF DMAs swap the halves into h.
                    nc.vector.tensor_scalar(
                        out=hv[:, 8 * bA + 1: 8 * bA + 9, 1:65],
                        in0=pc[:, :].rearrange("p (a b) -> p a b", b=64),
                        scalar1=b1[:, 0:1], scalar2=0.0,
                        op0=ALU_ADD, op1=ALU_GE)
                    nc.vector.tensor_scalar(
                        out=tb[:, r * 512:(r + 1) * 512], in0=pd[:, :],
                        scalar1=b1[:, 0:1], scalar2=0.0,
                        op0=ALU_ADD, op1=ALU_GE)
                    tbv = tb[:, r * 512:(r + 1) * 512].rearrange(
                        "p (a b) -> p a b", b=64)
                    nc.sync.dma_start(
                        out=hv[0:64, 8 * bB + 1: 8 * bB + 9, 1:65],
                        in_=tbv[64:128])
                    nc.sync.dma_start(
                        out=hv[64:128, 8 * bB + 1: 8 * bB + 9, 1:65],
                        in_=tbv[0:64])
                # halo rows between the two partition groups
                nc.sync.dma_start(out=hv[0:64, 33:34, 1:65], in_=hv[64:128, 1:2, 1:65])
                nc.sync.dma_start(out=hv[64:128, 0:1, 1:65], in_=hv[0:64, 32:33, 1:65])

            def emit_conv2(iv, last=False):
                img = iv % IMGS_PER_CORE
                buf = iv % nbuf
                hv = hs[buf].rearrange("p (r c) -> p r c", c=HP)
                for r in range(2):
                    bA, bB = 2 * r, 2 * r + 1
                    if dr:
                        pts = [pool2.tile([64, 512], F32, tag=f"d{i}",
                                          name=f"p2d{i}", bufs=1)
                               for i in range(4)]
                        # Order blocks so consecutive matmuls on the same
                        # row-tile share identical (duplicated g0/g1) weights
                        # - redundant LDWEIGHTS can be elided; the PE reorder
                        # window still runs the two row-groups concurrently.
                        blocks = ((0, bA), (0, bB), (1, bA), (1, bB))
                        n_mm = 0
                        for dx in range(3):
                            for pair in range(2):
                                dy0 = 2 * pair      # planes (0,1) or (2,zero)
                                st, sp = n_mm == 0, n_mm == 5
                                n_mm += 1
                                for ps, (g, blk) in zip(pts, blocks):
                                    lw = w2v[g * 64:(g + 1) * 64, dx, pair]
                                    sl = hv[g * 64:(g + 1) * 64,
                                            8 * blk + dy0: 8 * blk + dy0 + 8,
                                            dx: dx + 64]
                                    rhs = AP(tensor=sl.tensor, offset=sl.offset,
                                             ap=[list(sl.ap[0]), [HP, 2],
                                                 [HP, 8], [1, 64]])
                                    nc.tensor.matmul(
                                        ps[:, :], lw, rhs, start=st, stop=sp,
                                        perf_mode=DR, skip_group_check=True,
                                        tile_position=(g * 64, 0))
                        # Evacuate the 4 blocks into quarters of one SBUF
                        # tile ordered [g, blk], then write the whole round
                        # with a single 4D-AP DMA (out rows g*32 + 8*blk for
                        # g,blk in {0,1}x{bA,bB}) - 2 output DMAs per image
                        # instead of 8 cuts HWDGE descriptor-gen load and the
                        # end-of-kernel drain tail.
                        ob = ob_pool.tile([64, 2048], F16, tag="od", name="ob")
                        quarter = {(0, bA): 0, (0, bB): 1, (1, bA): 2, (1, bB): 3}
                        # all four on ScalarE: DVE owns the conv1 is_ge
                        # evacuations, so the engines stay balanced (~3.6us
                        # ACT vs ~2.7us DVE per image on real HW). The final
                        # round splits 2/2 across ACT+DVE so the end-of-
                        # kernel drain chain halves.
                        for i, (ps, (g, blk)) in enumerate(zip(pts, blocks)):
                            q = quarter[(g, blk)]
                            dst = ob[:, q * 512:(q + 1) * 512]
                            if last and r == 1 and i % 2 == 1:
                                nc.vector.tensor_scalar(
                                    out=dst, in0=ps[:, :],
                                    scalar1=2.0, scalar2=b2[0:64, 0:1],
                                    op0=ALU_MULT, op1=ALU_ADD)
                            else:
                                nc.scalar.activation(
                                    out=dst, in_=ps[:, :], scale=2.0,
                                    func=ACT_IDENT, bias=b2[0:64, 0:1])
                        base = 8 * bA * 64
                        sl = out_t.ap()[img][:, base: base + 512]
                        if last and r == 1:
                            # final round: two half-DMAs so the first leaves
                            # as soon as the g0 evacuations land - shortens
                            # the end-of-kernel drain.
                            for g in range(2):
                                slg = out_t.ap()[img][:, base + g * 2048:
                                                      base + g * 2048 + 512]
                                dg = AP(tensor=slg.tensor, offset=slg.offset,
                                        ap=[list(slg.ap[0]), [512, 2], [1, 512]])
                                nc.sync.dma_start(
                                    out=dg, in_=ob[:, g * 1024:(g + 1) * 1024])
                        else:
                            dram = AP(tensor=sl.tensor, offset=sl.offset,
                                      ap=[list(sl.ap[0]), [2048, 2], [512, 2],
                                          [1, 512]])
                            nc.sync.dma_start(out=dram, in_=ob[:, :])
                        continue
                    # non-DR fallback: plain fp8, 4-way col/row tiling
                    pe = pool2.tile([128, 512], F32, tag="c2a", name="pe")
                    pf = pool2.tile([128, 512], F32, tag="c2b", name="pf")
                    for tap in range(9):
                        dy, dx = divmod(tap, 3)
                        st, sp = tap == 0, tap == 8
                        for (ps, half, g, blk) in (
                                (pe, slice(0, 64), 0, bA),
                                (pe, slice(64, 128), 1, bA),
                                (pf, slice(64, 128), 0, bB),
                                (pf, slice(0, 64), 1, bB)):
                            lw = w2.rearrange("p (t m) -> p t m", m=64)[
                                g * 64:(g + 1) * 64, tap]
                            rhs = hv[g * 64:(g + 1) * 64,
                                     8 * blk + dy: 8 * blk + dy + 8,
                                     dx: dx + 64]
                            nc.tensor.matmul(
                                ps[half, :], lw, rhs, start=st, stop=sp,
                                skip_group_check=True,
                                tile_position=(g * 64, ps[half, :].base_partition()))
                    obe = ob_pool.tile([128, 512], F16, tag="obe", name="obe")
                    obf = ob_pool.tile([128, 512], F16, tag="obf", name="obf")
                    nc.scalar.activation(out=obe[:, :], in_=pe[:, :], scale=2.0,
                                         func=ACT_IDENT, bias=b2[:, 0:1])
                    nc.vector.tensor_scalar(out=obf[:, :], in0=pf[:, :],
                                            scalar1=2.0, scalar2=b2[:, 0:1],
                                            op0=ALU_MULT, op1=ALU_ADD)
                    for (ob, half, g, blk) in (
                            (obe, slice(0, 64), 0, bA),
                            (obe, slice(64, 128), 1, bA),
                            (obf, slice(64, 128), 0, bB),
                            (obf, slice(0, 64), 1, bB)):
                        row0 = g * 32 + 8 * blk
                        nc.sync.dma_start(
                            out=out_t.ap()[img, :, row0 * 64:(row0 + 8) * 64],
                            in_=ob[half, :])

            total = IMGS_PER_CORE * reps
            for iv in range(total + 1):
                if iv < total:
                    if iv + 1 < total:
                        load_x(iv + 1)
                    emit_conv1(iv)
                if iv >= 1:
                    emit_conv2(iv - 1, last=(iv == total))

    nc.compile()
    return nc


_CACHE: dict = {}


def _get_nc(scale_val=None, reps: int = 1, **kw) -> bacc.Bacc:
    key = (reps, tuple(sorted(kw.items())))
    if key not in _CACHE:
        _CACHE[key] = build_nc(reps, **kw)
    return _CACHE[key]


def _sign(a: np.ndarray) -> np.ndarray:
    return np.where(a >= 0, np.float32(1.0), np.float32(-1.0))


def _prep_inputs(x, w1, b1, w2, b2, scale_val, dr: bool = True):
    x = np.asarray(x, np.float32)
    n = x.shape[0]
    xhi = x.astype(np.float16)
    xlo = (x - xhi.astype(np.float32)).astype(np.float16)
    xpad = np.zeros((n, 2, CIN, H + 2, W + 2), np.float16)
    xpad[:, 0, :, 1:65, 1:65] = xhi
    xpad[:, 1, :, 1:65, 1:65] = xlo
    # xg[i, g*64 + half*32 + c, r, cc] = xpad[i, half, c, g*32 + r, cc]
    xg = np.zeros((n, 128, XROWS, XCOLS), np.float16)
    for g in range(2):
        sl = xpad[:, :, :, g * 32: g * 32 + XROWS, :]      # [n,2,32,34,66]
        xg[:, g * 64:(g + 1) * 64] = sl.reshape(n, 64, XROWS, XCOLS)
    xg = xg.reshape(n, 128, XFREE)

    w1b = _sign(np.asarray(w1, np.float32))                # [64o, 32c, 3, 3]
    w1s = np.zeros((128, 9, 64), np.float16)
    for tap in range(9):
        dy, dx = divmod(tap, 3)
        blk = w1b[:, :, dy, dx].T                          # [32c, 64o]
        w1s[0:32, tap] = blk
        w1s[32:64, tap] = blk
        w1s[64:96, tap] = blk
        w1s[96:128, tap] = blk
    w1s = w1s.reshape(128, 9 * 64)

    w2b = _sign(np.asarray(w2, np.float32))                # [64o, 64c, 3, 3]
    w2s = np.zeros((128, 768), ml_dtypes.float8_e4m3fn)
    if dr:
        w2v = w2s.reshape(128, 3, 2, 2, 64)
        for dx in range(3):
            for pair in range(2):
                for plane in range(2):
                    dy = 2 * pair + plane
                    if dy <= 2:
                        blk = w2b[:, :, dy, dx].T.astype(ml_dtypes.float8_e4m3fn)
                        w2v[0:64, dx, pair, plane] = blk
                        w2v[64:128, dx, pair, plane] = blk
    else:
        w2v = w2s[:, :576].reshape(128, 9, 64)
        for tap in range(9):
            dy, dx = divmod(tap, 3)
            blk = w2b[:, :, dy, dx].T.astype(ml_dtypes.float8_e4m3fn)
            w2v[0:64, tap] = blk
            w2v[64:128, tap] = blk

    b1s = np.tile(_sign(np.asarray(b1, np.float32)), 2).reshape(128, 1).astype(np.float32)
    # h is stored as h01 (see build_nc): out = 2*conv2(w, h01) - sum(w2) + sign(b2)
    t_oc = w2b.sum(axis=(1, 2, 3))                         # [64]
    b2s = np.tile(_sign(np.asarray(b2, np.float32)) - t_oc, 2).reshape(128, 1).astype(np.float32)

    per = n // N_CORES
    in_maps = []
    for i in range(N_CORES):
        sl = slice(i * per, (i + 1) * per)
        in_maps.append({
            "xg": np.ascontiguousarray(xg[sl]),
            "w1s": w1s, "w2s": w2s, "b1s": b1s, "b2s": b2s,
        })
    return in_maps


def kernel(x, w1, b1, w2, b2, scale) -> np.ndarray:
    scale_val = float(np.asarray(scale).reshape(-1)[0])
    nc = _get_nc(reps=1)
    in_maps = _prep_inputs(x, w1, b1, w2, b2, scale_val)
    res = run_bass_kernel_spmd(nc, in_maps, core_ids=list(range(N_CORES)))
    out16 = np.concatenate([r["out"] for r in res.results], axis=0)
    out = out16.astype(np.float32) * np.float32(scale_val)
    return out.reshape(-1, COUT, H, W)


if __name__ == "__main__":
    rng = np.random.default_rng(0)
    ins = {
        "x": rng.standard_normal((64, 32, 64, 64), dtype=np.float32),
        "w1": (rng.standard_normal((64, 32, 3, 3)) * 0.05).astype(np.float32),
        "b1": (rng.standard_normal((64,)) * 0.05).astype(np.float32),
        "w2": (rng.standard_normal((64, 64, 3, 3)) * 0.05).astype(np.float32),
        "b2": (rng.standard_normal((64,)) * 0.05).astype(np.float32),
        "scale": np.array([0.001], np.float32),
    }
    out = kernel(**ins)
    print("out", out.shape, out.dtype, float(np.abs(out).mean()))


# revision 33
# speedup vs baseline: 1.0036x; 1.0036x over previous
"""Binarized 2-layer conv net (BinaryConv2d -> BinaryTanh -> BinaryConv2d -> Scale)
for Trainium2, data-parallel over the batch dim across 8 NeuronCores.

Math (matching the reference):
    h   = conv2d(x, sign(w1), pad=1) + sign(b1)
    h   = sign(h)                       # sign(clip(h,-1,1)) == sign(h)
    out = (conv2d(h, sign(w2), pad=1) + sign(b2)) * scale

Per-core design (8 images):
  * x is hi/lo fp16 split (hi+lo ~ 24 mantissa bits, effectively fp32-exact)
    folded into the matmul K dim: partitions g*64 + {0:32 hi, 32:64 lo} for
    row-group g; weights duplicated across hi/lo halves. One copy of x in
    SBUF per image ([128, 34, 66] fp16, 2 row-groups of 32 output rows each
    with halo rows), 9 conv taps read via shifted access patterns.
  * conv1: K=64, M=64, 9 tap-matmuls per 512-px block, 4 blocks concurrent
    on the four 64x64 PE tiles. Evacuation is VectorE (psum + b1) >= 0 into
    the fp8 h staging buffer as h01 in {0,1} (0.5 at padding borders), so
    ScalarE stays off the critical path; conv2 compensates exactly with
    out = 2*psum - sum(w2 signs) + sign(b2).
  * h staging: [128 = 2 row-groups x 64 ch, 35 rows x 80 cols] fp8e4 with
    0.5 borders + halo rows; 80B row pitch makes dy-shifted tap pairs
    16B-aligned for DoubleRow.
  * conv2: fp8e4 DoubleRow - per dx, taps (dy0,dy1) form one double-row
    matmul and (dy2, zero-weights) a second: 6 half-rate matmuls per block
    instead of 9 full-rate. {0,0.5,1} are exact in fp8e4, products
    accumulate in fp32 psum, so conv2 is exact. DoubleRow outputs must land
    at psum partition base 0 (col tiling is ISA-incompatible with DR), so
    each block gets its own [64, 512] psum tile; concurrency comes from the
    two K row-groups, with same-weight matmuls adjacent for LDW elision.
    Per round, the four evacuations (ScalarE, scale=2, bias) fill one
    [64, 2048] tile written by a single 4D-AP output DMA.
  * Software pipeline: iteration i emits conv1(i) then conv2(i-1), with the
    x DMA for image i+1 prefetched at the top - conv2(i-1)'s dependencies
    (conv1(i-1) evacuations + halo DMAs) complete while the PE runs
    conv1(i), so the PE never stalls at the conv1->conv2 boundary.
  * Output: psum + sign(b2) are exact small integers -> written as fp16
    (exact to 2048), DMA'd at half the bytes; host multiplies by scale in
    fp32 (bit-identical to doing it on-device).
"""

import numpy as np
import ml_dtypes

import concourse.mybir as mybir
import concourse.tile as tile
from concourse import bacc
from concourse.ap import AP
from concourse.bass_utils import run_bass_kernel_spmd

F32 = mybir.dt.float32
F16 = mybir.dt.float16
FP8 = mybir.dt.float8e4
ACT_SIGN = mybir.ActivationFunctionType.Sign
ACT_IDENT = mybir.ActivationFunctionType.Identity
ALU_MULT = mybir.AluOpType.mult
ALU_ADD = mybir.AluOpType.add
ALU_GE = mybir.AluOpType.is_ge
DR = mybir.MatmulPerfMode.DoubleRow

N_CORES = 8
IMGS_PER_CORE = 8
CIN, COUT = 32, 64
H = W = 64
XROWS, XCOLS = 34, 66          # per-group x slab: 32 out rows + 2 halo
XFREE = XROWS * XCOLS          # 2244
HROWS, HP = 35, 80             # h slab: 34 rows + 1 zero overrun, 80B pitch
HFREE = HROWS * HP             # 2800


def build_nc(reps: int = 1, nbuf: int = 2, dr: bool = True) -> bacc.Bacc:
    nc = bacc.Bacc("TRN2", target_bir_lowering=False)

    xg_t = nc.dram_tensor("xg", [IMGS_PER_CORE, 128, XFREE], F16, kind="ExternalInput")
    w1s_t = nc.dram_tensor("w1s", [128, 9 * 64], F16, kind="ExternalInput")
    w2s_t = nc.dram_tensor("w2s", [128, 768], FP8, kind="ExternalInput")
    b1s_t = nc.dram_tensor("b1s", [128, 1], F32, kind="ExternalInput")
    b2s_t = nc.dram_tensor("b2s", [128, 1], F32, kind="ExternalInput")
    out_t = nc.dram_tensor("out", [IMGS_PER_CORE, COUT, H * W], F16, kind="ExternalOutput")

    with tile.TileContext(nc) as tc:
        w1 = nc.alloc_sbuf_tensor("w1sb", [128, 9 * 64], F16).ap()
        w2 = nc.alloc_sbuf_tensor("w2sb", [128, 768], FP8).ap()
        b1 = nc.alloc_sbuf_tensor("b1sb", [128, 1], F32).ap()
        b2 = nc.alloc_sbuf_tensor("b2sb", [128, 1], F32).ap()
        xs = [nc.alloc_sbuf_tensor(f"xs_{b}", [128, XFREE], F16).ap()
              for b in range(nbuf)]
        hs = [nc.alloc_sbuf_tensor(f"hs_{b}", [128, HFREE], FP8).ap()
              for b in range(nbuf)]

        # x loads split at row 18 (round 0 of conv1 reads rows 0..17, round 1
        # reads 16..33) so the first matmuls only wait for half the 574KB.
        XSPLIT = 18 * XCOLS

        def load_x(iv):
            # First half rides the Activation HWDGE ring (ScalarE-issued),
            # second half the SP ring - the two complete in parallel, so
            # round 0's matmuls (gated on the first half) start sooner.
            buf, img = iv % nbuf, iv % IMGS_PER_CORE
            nc.scalar.dma_start(out=xs[buf][:, :XSPLIT],
                                in_=xg_t.ap()[img][:, :XSPLIT])
            nc.sync.dma_start(out=xs[buf][:, XSPLIT:],
                              in_=xg_t.ap()[img][:, XSPLIT:])

        # Weights first (land fast, gate the first Ldweights), then the
        # first x image; x rides the parallel Activation ring. Tap-0 of w1
        # ships separately (16KB) so the very first Ldweights doesn't wait
        # for the full 147KB w1 transfer.
        nc.sync.dma_start(out=w1[:, 0:64], in_=w1s_t.ap()[:, 0:64])
        nc.sync.dma_start(out=w1[:, 64:], in_=w1s_t.ap()[:, 64:])
        nc.sync.dma_start(out=w2, in_=w2s_t.ap())
        nc.sync.dma_start(out=b1, in_=b1s_t.ap())
        nc.sync.dma_start(out=b2, in_=b2s_t.ap())
        load_x(0)
        # h is stored as h01 = (conv1+b1 >= 0) in {0,1}; padding borders hold
        # 0.5 so that h = 2*h01 - 1 is 0 there, and conv2 compensates with
        # out = 2*psum - sum(w2) + sign(b2). This lets conv1 evacuate via a
        # single DVE add/is_ge instead of ScalarE Sign, taking the Activation
        # engine off the real-HW critical path. Border-only memsets (rows
        # 0/33/34, cols 0/65): the interior is overwritten every image and
        # the 66:80 column pad is never read.
        for b in range(nbuf):
            hvb = hs[b].rearrange("p (r c) -> p r c", c=HP)
            nc.vector.memset(hvb[:, 0:1, :], 0.5)
            nc.vector.memset(hvb[:, 33:35, :], 0.5)
            nc.vector.memset(hvb[:, :, 0:1], 0.5)
            nc.vector.memset(hvb[:, :, 65:66], 0.5)

        w2v = w2.rearrange("p (dx pair plane m) -> p dx pair plane m",
                           dx=3, pair=2, plane=2)

        warm = nc.alloc_sbuf_tensor("warm", [64, 512], F16).ap()
        nc.vector.memset(warm[:, :], 0.0)

        with tc.tile_pool(name="ps1", bufs=2, space="PSUM") as pool1, \
             tc.tile_pool(name="ps2", bufs=2, space="PSUM") as pool2, \
             tc.tile_pool(name="ob", bufs=4) as ob_pool:
            # Warm-up matmuls on zeroed dummy data during the startup DMA
            # wait: the PE clock gate (HAM) and the cost model's p-state ramp
            # need ~3us of sustained activity to reach full speed - start
            # that clock while the weights/x loads are still in flight, so
            # the first real matmuls run at 2.4GHz instead of ramping.
            for wi in range(4):
                pw = pool1.tile([128, 512], F32, tag="c1a", name="pw")
                nc.tensor.matmul(pw[0:64, :], warm[:, 0:64], warm[:, :],
                                 start=True, stop=True, tile_position=(0, 0),
                                 skip_group_check=True)

            def emit_conv1(iv):
                buf = iv % nbuf
                xv = xs[buf].rearrange("p (r c) -> p r c", c=XCOLS)
                hv = hs[buf].rearrange("p (r c) -> p r c", c=HP)
                tb = ob_pool.tile([128, 1024], FP8, tag="tb", name="tb")
                for r in range(2):
                    bA, bB = 2 * r, 2 * r + 1   # block = 8 output rows
                    pc = pool1.tile([128, 512], F32, tag="c1a", name="pc")
                    pd = pool1.tile([128, 512], F32, tag="c1b", name="pd")
                    for tap in range(9):
                        dy, dx = divmod(tap, 3)
                        st, sp = tap == 0, tap == 8
                        lw0 = w1[0:64, tap * 64:(tap + 1) * 64]
                        lw1 = w1[64:128, tap * 64:(tap + 1) * 64]
                        rA0 = xv[0:64, 8 * bA + dy: 8 * bA + dy + 8, dx: dx + 64]
                        rA1 = xv[64:128, 8 * bA + dy: 8 * bA + dy + 8, dx: dx + 64]
                        rB0 = xv[0:64, 8 * bB + dy: 8 * bB + dy + 8, dx: dx + 64]
                        rB1 = xv[64:128, 8 * bB + dy: 8 * bB + dy + 8, dx: dx + 64]
                        nc.tensor.matmul(pc[0:64, :], lw0, rA0, start=st, stop=sp,
                                         tile_position=(0, 0), skip_group_check=True)
                        nc.tensor.matmul(pc[64:128, :], lw1, rA1, start=st, stop=sp,
                                         tile_position=(64, 64), skip_group_check=True)
                        nc.tensor.matmul(pd[64:128, :], lw0, rB0, start=st, stop=sp,
                                         tile_position=(0, 64), skip_group_check=True)
                        nc.tensor.matmul(pd[0:64, :], lw1, rB1, start=st, stop=sp,
                                         tile_position=(64, 0), skip_group_check=True)
                    # h[g, out rows of block, 1:65] = sign(psum + b1).
                    # pc halves are group-aligned -> one direct ACT. pd is
                    # crosswise (tiles (0,64)/(64,0) force the psum column
                    # half) -> ACT into a temp, then two partition-crossing
                    # SBUF DMAs swap the halves into h.
                    nc.vector.tensor_scalar(
                        out=hv[:, 8 * bA + 1: 8 * bA + 9, 1:65],
                        in0=pc[:, :].rearrange("p (a b) -> p a b", b=64),
                        scalar1=b1[:, 0:1], scalar2=0.0,
                        op0=ALU_ADD, op1=ALU_GE)
                    nc.vector.tensor_scalar(
                        out=tb[:, r * 512:(r + 1) * 512], in0=pd[:, :],
                        scalar1=b1[:, 0:1], scalar2=0.0,
                        op0=ALU_ADD, op1=ALU_GE)
                    tbv = tb[:, r * 512:(r + 1) * 512].rearrange(
                        "p (a b) -> p a b", b=64)
                    nc.sync.dma_start(
                        out=hv[0:64, 8 * bB + 1: 8 * bB + 9, 1:65],
                        in_=tbv[64:128])
                    nc.sync.dma_start(
                        out=hv[64:128, 8 * bB + 1: 8 * bB + 9, 1:65],
                        in_=tbv[0:64])
                # halo rows between the two partition groups
                nc.sync.dma_start(out=hv[0:64, 33:34, 1:65], in_=hv[64:128, 1:2, 1:65])
                nc.sync.dma_start(out=hv[64:128, 0:1, 1:65], in_=hv[0:64, 32:33, 1:65])

            def emit_conv2(iv, last=False):
                img = iv % IMGS_PER_CORE
                buf = iv % nbuf
                hv = hs[buf].rearrange("p (r c) -> p r c", c=HP)
                for r in range(2):
                    bA, bB = 2 * r, 2 * r + 1
                    if dr:
                        pts = [pool2.tile([64, 512], F32, tag=f"d{i}",
                                          name=f"p2d{i}", bufs=1)
                               for i in range(4)]
                        # Order blocks so consecutive matmuls on the same
                        # row-tile share identical (duplicated g0/g1) weights
                        # - redundant LDWEIGHTS can be elided; the PE reorder
                        # window still runs the two row-groups concurrently.
                        blocks = ((0, bA), (0, bB), (1, bA), (1, bB))
                        n_mm = 0
                        for dx in range(3):
                            for pair in range(2):
                                dy0 = 2 * pair      # planes (0,1) or (2,zero)
                                st, sp = n_mm == 0, n_mm == 5
                                n_mm += 1
                                for ps, (g, blk) in zip(pts, blocks):
                                    lw = w2v[g * 64:(g + 1) * 64, dx, pair]
                                    sl = hv[g * 64:(g + 1) * 64,
                                            8 * blk + dy0: 8 * blk + dy0 + 8,
                                            dx: dx + 64]
                                    rhs = AP(tensor=sl.tensor, offset=sl.offset,
                                             ap=[list(sl.ap[0]), [HP, 2],
                                                 [HP, 8], [1, 64]])
                                    nc.tensor.matmul(
                                        ps[:, :], lw, rhs, start=st, stop=sp,
                                        perf_mode=DR, skip_group_check=True,
                                        tile_position=(g * 64, 0))
                        # Evacuate the 4 blocks into quarters of one SBUF
                        # tile ordered [g, blk], then write the whole round
                        # with a single 4D-AP DMA (out rows g*32 + 8*blk for
                        # g,blk in {0,1}x{bA,bB}) - 2 output DMAs per image
                        # instead of 8 cuts HWDGE descriptor-gen load and the
                        # end-of-kernel drain tail.
                        ob = ob_pool.tile([64, 2048], F16, tag="od", name="ob")
                        quarter = {(0, bA): 0, (0, bB): 1, (1, bA): 2, (1, bB): 3}
                        # all four on ScalarE: DVE owns the conv1 is_ge
                        # evacuations, so the engines stay balanced (~3.6us
                        # ACT vs ~2.7us DVE per image on real HW). The final
                        # round splits 2/2 across ACT+DVE so the end-of-
                        # kernel drain chain halves.
                        for i, (ps, (g, blk)) in enumerate(zip(pts, blocks)):
                            q = quarter[(g, blk)]
                            dst = ob[:, q * 512:(q + 1) * 512]
                            if last and r == 1 and i % 2 == 1:
                                nc.vector.tensor_scalar(
                                    out=dst, in0=ps[:, :],
                                    scalar1=2.0, scalar2=b2[0:64, 0:1],
                                    op0=ALU_MULT, op1=ALU_ADD)
                            else:
                                nc.scalar.activation(
                                    out=dst, in_=ps[:, :], scale=2.0,
                                    func=ACT_IDENT, bias=b2[0:64, 0:1])
                        base = 8 * bA * 64
                        sl = out_t.ap()[img][:, base: base + 512]
                        if last and r == 1:
                            # final round: two half-DMAs so the first leaves
                            # as soon as the g0 evacuations land - shortens
                            # the end-of-kernel drain.
                            for g in range(2):
                                slg = out_t.ap()[img][:, base + g * 2048:
                                                      base + g * 2048 + 512]
                                dg = AP(tensor=slg.tensor, offset=slg.offset,
                                        ap=[list(slg.ap[0]), [512, 2], [1, 512]])
                                nc.sync.dma_start(
                                    out=dg, in_=ob[:, g * 1024:(g + 1) * 1024])
                        else:
                            dram = AP(tensor=sl.tensor, offset=sl.offset,
                                      ap=[list(sl.ap[0]), [2048, 2], [512, 2],
                                          [1, 512]])
                            nc.sync.dma_start(out=dram, in_=ob[:, :])
                        continue
                    # non-DR fallback: plain fp8, 4-way col/row tiling
                    pe = pool2.tile([128, 512], F32, tag="c2a", name="pe")
                    pf = pool2.tile([128, 512], F32, tag="c2b", name="pf")
                    for tap in range(9):
                        dy, dx = divmod(tap, 3)
                        st, sp = tap == 0, tap == 8
                        for (ps, half, g, blk) in (
                                (pe, slice(0, 64), 0, bA),
                                (pe, slice(64, 128), 1, bA),
                                (pf, slice(64, 128), 0, bB),
                                (pf, slice(0, 64), 1, bB)):
                            lw = w2.rearrange("p (t m) -> p t m", m=64)[
                                g * 64:(g + 1) * 64, tap]
                            rhs = hv[g * 64:(g + 1) * 64,
                                     8 * blk + dy: 8 * blk + dy + 8,
                                     dx: dx + 64]
                            nc.tensor.matmul(
                                ps[half, :], lw, rhs, start=st, stop=sp,
                                skip_group_check=True,
                                tile_position=(g * 64, ps[half, :].base_partition()))
                    obe = ob_pool.tile([128, 512], F16, tag="obe", name="obe")
                    obf = ob_pool.tile([128, 512], F16, tag="obf", name="obf")
                    nc.scalar.activation(out=obe[:, :], in_=pe[:, :], scale=2.0,
                                         func=ACT_IDENT, bias=b2[:, 0:1])
                    nc.vector.tensor_scalar(out=obf[:, :], in0=pf[:, :],
                                            scalar1=2.0, scalar2=b2[:, 0:1],
                                            op0=ALU_MULT, op1=ALU_ADD)
                    for (ob, half, g, blk) in (
                            (obe, slice(0, 64), 0, bA),
                            (obe, slice(64, 128), 1, bA),
                            (obf, slice(64, 128), 0, bB),
                            (obf, slice(0, 64), 1, bB)):
                        row0 = g * 32 + 8 * blk
                        nc.sync.dma_start(
                            out=out_t.ap()[img, :, row0 * 64:(row0 + 8) * 64],
                            in_=ob[half, :])

            total = IMGS_PER_CORE * reps
            for iv in range(total + 1):
                if iv < total:
                    if iv + 1 < total:
                        load_x(iv + 1)
                    emit_conv1(iv)
                if iv >= 1:
                    emit_conv2(iv - 1, last=(iv == total))

    nc.compile()
    return nc


_CACHE: dict = {}


def _get_nc(scale_val=None, reps: int = 1, **kw) -> bacc.Bacc:
    key = (reps, tuple(sorted(kw.items())))
    if key not in _CACHE:
        _CACHE[key] = build_nc(reps, **kw)
    return _CACHE[key]


def _sign(a: np.ndarray) -> np.ndarray:
    return np.where(a >= 0, np.float32(1.0), np.float32(-1.0))


def _prep_inputs(x, w1, b1, w2, b2, scale_val, dr: bool = True):
    x = np.asarray(x, np.float32)
    n = x.shape[0]
    xhi = x.astype(np.float16)
    xlo = (x - xhi.astype(np.float32)).astype(np.float16)
    xpad = np.zeros((n, 2, CIN, H + 2, W + 2), np.float16)
    xpad[:, 0, :, 1:65, 1:65] = xhi
    xpad[:, 1, :, 1:65, 1:65] = xlo
    # xg[i, g*64 + half*32 + c, r, cc] = xpad[i, half, c, g*32 + r, cc]
    xg = np.zeros((n, 128, XROWS, XCOLS), np.float16)
    for g in range(2):
        sl = xpad[:, :, :, g * 32: g * 32 + XROWS, :]      # [n,2,32,34,66]
        xg[:, g * 64:(g + 1) * 64] = sl.reshape(n, 64, XROWS, XCOLS)
    xg = xg.reshape(n, 128, XFREE)

    w1b = _sign(np.asarray(w1, np.float32))                # [64o, 32c, 3, 3]
    w1s = np.zeros((128, 9, 64), np.float16)
    for tap in range(9):
        dy, dx = divmod(tap, 3)
        blk = w1b[:, :, dy, dx].T                          # [32c, 64o]
        w1s[0:32, tap] = blk
        w1s[32:64, tap] = blk
        w1s[64:96, tap] = blk
        w1s[96:128, tap] = blk
    w1s = w1s.reshape(128, 9 * 64)

    w2b = _sign(np.asarray(w2, np.float32))                # [64o, 64c, 3, 3]
    w2s = np.zeros((128, 768), ml_dtypes.float8_e4m3fn)
    if dr:
        w2v = w2s.reshape(128, 3, 2, 2, 64)
        for dx in range(3):
            for pair in range(2):
                for plane in range(2):
                    dy = 2 * pair + plane
                    if dy <= 2:
                        blk = w2b[:, :, dy, dx].T.astype(ml_dtypes.float8_e4m3fn)
                        w2v[0:64, dx, pair, plane] = blk
                        w2v[64:128, dx, pair, plane] = blk
    else:
        w2v = w2s[:, :576].reshape(128, 9, 64)
        for tap in range(9):
            dy, dx = divmod(tap, 3)
            blk = w2b[:, :, dy, dx].T.astype(ml_dtypes.float8_e4m3fn)
            w2v[0:64, tap] = blk
            w2v[64:128, tap] = blk

    b1s = np.tile(_sign(np.asarray(b1, np.float32)), 2).reshape(128, 1).astype(np.float32)
    # h is stored as h01 (see build_nc): out = 2*conv2(w, h01) - sum(w2) + sign(b2)
    t_oc = w2b.sum(axis=(1, 2, 3))                         # [64]
    b2s = np.tile(_sign(np.asarray(b2, np.float32)) - t_oc, 2).reshape(128, 1).astype(np.float32)

    per = n // N_CORES
    in_maps = []
    for i in range(N_CORES):
        sl = slice(i * per, (i + 1) * per)
        in_maps.append({
            "xg": np.ascontiguousarray(xg[sl]),
            "w1s": w1s, "w2s": w2s, "b1s": b1s, "b2s": b2s,
        })
    return in_maps


def kernel(x, w1, b1, w2, b2, scale) -> np.ndarray:
    scale_val = float(np.asarray(scale).reshape(-1)[0])
    nc = _get_nc(reps=1)
    in_maps = _prep_inputs(x, w1, b1, w2, b2, scale_val)
    res = run_bass_kernel_spmd(nc, in_maps, core_ids=list(range(N_CORES)))
    out16 = np.concatenate([r["out"] for r in res.results], axis=0)
    out = out16.astype(np.float32) * np.float32(scale_val)
    return out.reshape(-1, COUT, H, W)


if __name__ == "__main__":
    rng = np.random.default_rng(0)
    ins = {
        "x": rng.standard_normal((64, 32, 64, 64), dtype=np.float32),
        "w1": (rng.standard_normal((64, 32, 3, 3)) * 0.05).astype(np.float32),
        "b1": (rng.standard_normal((64,)) * 0.05).astype(np.float32),
        "w2": (rng.standard_normal((64, 64, 3, 3)) * 0.05).astype(np.float32),
        "b2": (rng.standard_normal((64,)) * 0.05).astype(np.float32),
        "scale": np.array([0.001], np.float32),
    }
    out = kernel(**ins)
    print("out", out.shape, out.dtype, float(np.abs(out).mean()))
